# revision 22
# baseline (speedup 1.0000x reference)
"""CQAttention Bass kernel for TRN2, 8 NeuronCores, batch-parallel.

Problem shapes (hardcoded): context [16,128,2048] f32, query [16,128,512] f32,
w [384] f32 -> out [16,512,2048] f32.

Math per batch (D=128, C=2048, Q=512):
  s[c,q]  = bias_c[c] + bias_q[q] + sum_d ctx[d,c]*wcq[d]*qry[d,q]
  s1      = softmax_c(s)            (bias_q is constant along c -> cancels)
  a       = s1 @ qry^T-ish          aT[d,c] = sum_q qryT[q,d] s1T[q,c]
  t[q,d]  = sum_c s1[c,q] ctx[d,c]  (s1^T @ ctx)
  b2T     = sum_q t2[q,d] s1T[q,c]  (associativity: s1@(s1^T@ctx), avoids [C,C])
  out     = [ctxT; aT; ctxT*aT; ctxT*b2T]   ([4D, C] per batch)

Tricks:
  - bias_c folded into the moving operand: s+bias_c = ctx^T @ (qry*wcq + wc)
  - softmax max-subtraction skipped (scores are O(1); exp is safe in fp32)
  - S[q] = sum_c E[c,q] obtained free as column 128 of the t_raw matmul by
    augmenting ctxC with a ones column
  - 1/S folded into ET during the PSUM->SBUF copy (per-partition scalar)
  - fp32r (free bitcast) for fp32 matmuls with N>=256; bf16 elsewhere
"""

import numpy as np

import concourse.bass as bass
import concourse.mybir as mybir
import concourse.tile as tile
from concourse.bass import ts, ds
from concourse.bass_utils import run_bass_kernel_spmd
from concourse.masks import make_identity

B, D, C, Q = 16, 128, 2048, 512
NCORES = 8
BPC = B // NCORES          # batches per core
NCT = C // 128             # 16 c-tiles
NQT = Q // 128             # 4 q-tiles
NCH = C // 512             # 4 c-chunks
F32 = mybir.dt.float32
F32R = mybir.dt.float32r
BF16 = mybir.dt.bfloat16
AF = mybir.ActivationFunctionType


_SPLIT_TYPES = (
    "InstMatmult", "InstLdweights", "InstActivation", "InstTensorScalar",
    "InstTensorScalarPtr", "InstTensorScalarAffineSelect", "InstTensorTensor",
    "InstTensorCopy", "InstReciprocal", "InstMemset", "InstCopyPredicated",
    "InstBNStats", "InstStreamTranspose", "InstTensorReduce", "InstIota",
    "InstDMACopy", "InstDMA", "InstDMAGather", "InstDMAGatherAnt",
    "InstDrain",
)


def _split_multi_waits(nc, max_embedded=1):
    """walrus allows very few embedded sync-waits per compute instruction
    (AP-parameterized ops seem to have just one slot). Hoist extra waits
    into standalone event-semaphore instructions on the same engine."""
    n = 0
    for fn in nc.m.functions:
        for blk in fn.blocks:
            il = blk.instructions
            i = 0
            while i < len(il):
                inst = il[i]
                si = inst.sync_info
                if (si is not None and si.on_wait
                        and len(si.on_wait) > max_embedded
                        and type(inst).__name__ in _SPLIT_TYPES):
                    waits = list(si.on_wait)
                    extra, keep = waits[:-max_embedded], waits[-max_embedded:]
                    for k, w in enumerate(extra):
                        nop = mybir.InstEventSemaphore(
                            name=f"{inst.name}-w{k}", engine=inst.engine,
                            ins=[], outs=[])
                        nop.sync_info = mybir.SyncInfo(on_wait=[w],
                                                       on_update=[])
                        il.insert(i, nop)
                        i += 1
                        n += 1
                    inst.sync_info = mybir.SyncInfo(on_wait=keep,
                                                    on_update=si.on_update)
                i += 1
    return n


def build_kernel():
    nc = bass.Bass("TRN2", target_bir_lowering=False, debug=False,
                   num_devices=NCORES)
    ctx_ext = nc.dram_tensor("context", [BPC, D, C], F32,
                             kind="ExternalInput").ap()
    qry_ext = nc.dram_tensor("query", [BPC, D, Q], F32,
                             kind="ExternalInput").ap()
    w_ext = nc.dram_tensor("w", [3 * D], F32, kind="ExternalInput").ap()
    out_ext = nc.dram_tensor("out", [BPC, 4 * D, C], F32,
                             kind="ExternalOutput").ap()

    with tile.TileContext(nc) as tc:
        import contextlib
        with contextlib.ExitStack() as ex:
            singles = ex.enter_context(tc.tile_pool(name="singles", bufs=1))
            bb = ex.enter_context(tc.tile_pool(name="bb", bufs=2))
            stg = ex.enter_context(tc.tile_pool(name="stg", bufs=4))
            ps_pool = ex.enter_context(
                tc.tile_pool(name="ps", bufs=2, space="PSUM"))
            tr_pool = ex.enter_context(
                tc.tile_pool(name="tr", bufs=2, space="PSUM"))
            ab_pool = ex.enter_context(
                tc.tile_pool(name="ab", bufs=2, space="PSUM"))

            # ---- constants ----
            ident_bf = singles.tile([128, 128], BF16)
            make_identity(nc, ident_bf)
            # wcols[:, 0] = wc, wcols[:, 1] = wcq — one DMA; staged through a
            # DVE copy so consumers carry an engine-sem dep, not a second
            # HWDGE-sem dep (walrus allows only one HWDGE wait per instr).
            wcols_raw = singles.tile([128, 2], F32)
            nc.sync.dma_start(
                out=wcols_raw,
                in_=w_ext[ds(D, 2 * D)].rearrange("(o p) -> p o", o=2))
            wcols = singles.tile([128, 2], F32)
            nc.vector.tensor_copy(wcols, wcols_raw)
            wc_col = wcols[:, 0:1]
            wcq_col = wcols[:, 1:2]

            for b in range(BPC):
                # ---- loads (qry first — it gates the s-matmul rhs chain;
                # ctx chunked so downstream PE work starts early; all output
                # stores go on the gpsimd DMA queue to keep loads unblocked)
                qry_sb = bb.tile([128, Q], F32, tag="qry")
                ctx_sb = bb.tile([128, C], F32, tag="ctx")
                ctx_bf = bb.tile([128, C], BF16, tag="ctxbf")
                with tc.high_priority():
                    nc.sync.dma_start(out=qry_sb, in_=qry_ext[b])
                    for jl in range(NCH):
                        sl = ts(jl, 512)
                        nc.sync.dma_start(out=ctx_sb[:, sl],
                                          in_=ctx_ext[b][:, sl])
                for jl in range(NCH):
                    sl = ts(jl, 512)
                    nc.gpsimd.tensor_copy(ctx_bf[:, sl], ctx_sb[:, sl])
                    # section 1: ctx passthrough
                    nc.sync.dma_start(out=out_ext[b, 0:D, sl],
                                      in_=ctx_sb[:, sl])

                # qry in bf16 + its transpose (lhsT for the a-matmul)
                qry_bf = bb.tile([128, Q], BF16, tag="qrybf")
                nc.gpsimd.tensor_copy(qry_bf, qry_sb)

                # qryW2 = qry*wcq + wc   (bias_c folded into the matmul rhs).
                # Reads the DVE-produced qry_bf so all deps are one DVE sem —
                # ACTIVATE with two AP params has only one wait slot.
                qryW2 = bb.tile([128, Q], BF16, tag="qryW2")
                nc.scalar.activation(qryW2, qry_bf, AF.Identity,
                                     bias=wc_col, scale=wcq_col)
                p_qt = tr_pool.tile([128, 1024], BF16, tag="tr")
                for jq in range(NQT):
                    nc.tensor.transpose(
                        p_qt[:, ts(jq, 128)], qry_bf[:, ts(jq, 128)], ident_bf)
                qryT = bb.tile([128, NQT, 128], BF16, tag="qryT")
                nc.vector.tensor_copy(
                    qryT, p_qt[:, 0:512].rearrange("p (j d) -> p j d", j=NQT))

                # ctxC: ctx transposed to [C-part, D], bf16, with ones column
                # layout [128, 16, 132]: [:, jc, 0:128] data, [:, jc, 128] ones
                ctxC = bb.tile([128, NCT, 132], BF16, tag="ctxC")
                nc.vector.memset(ctxC[:, :, 128:129], 1.0)
                for jg in range(2):   # groups of 8 c-tiles per bf16 psum buf
                    p_ct = tr_pool.tile([128, 1024], BF16, tag="tr")
                    for jj in range(8):
                        jc = jg * 8 + jj
                        nc.tensor.transpose(
                            p_ct[:, ts(jj, 128)],
                            ctx_bf[:, ts(jc, 128)], ident_bf)
                    nc.vector.tensor_copy(
                        ctxC[:, ds(jg * 8, 8), 0:128],
                        p_ct.rearrange("p (j d) -> p j d", j=8))

                # ---- s-matmuls + exp ----
                E_sb = bb.tile([128, NCT, Q], BF16, tag="E")
                for jc in range(NCT):
                    p_s = ps_pool.tile([128, 512], F32, tag="ps")
                    nc.tensor.matmul(
                        p_s, lhsT=ctx_bf[:, ts(jc, 128)],
                        rhs=qryW2, start=True, stop=True)
                    nc.scalar.activation(E_sb[:, jc, :], p_s, AF.Exp)

                # ---- t_raw (+ S in col 128) ----
                # psum[q,0:129] = sum_c E[c,q-tile]^T @ [ctxC | 1]
                recip = bb.tile([128, NQT, 1], F32, tag="recip")
                t2 = bb.tile([128, NQT, 128], BF16, tag="t2")
                qryR = bb.tile([128, NQT, 128], BF16, tag="qryR")
                for jq in range(NQT):
                    p_t = tr_pool.tile([128, 512], F32, tag="tr")
                    for jc in range(NCT):
                        nc.tensor.matmul(
                            p_t[:, 0:129],
                            lhsT=E_sb[:, jc, ts(jq, 128)],
                            rhs=ctxC[:, jc, 0:129],
                            start=(jc == 0), stop=(jc == NCT - 1))
                    nc.vector.reciprocal(recip[:, jq, :], p_t[:, 128:129])
                    # recip is folded into the small lhsT tensors (qryR, t2)
                    # so the big ET copies below are not gated on it.
                    # t2 = t_raw * recip^2 ; qryR = qryT * recip
                    nc.vector.tensor_scalar(
                        out=t2[:, jq, :], in0=p_t[:, 0:128],
                        scalar1=recip[:, jq, :], scalar2=recip[:, jq, :],
                        op0=mybir.AluOpType.mult, op1=mybir.AluOpType.mult)
                    nc.vector.tensor_scalar_mul(
                        qryR[:, jq, :], qryT[:, jq, :], recip[:, jq, :])

                # ---- E transposes -> ET (unscaled; frees psum fast) ----
                ET = bb.tile([128, NQT, C], BF16, tag="ET")
                for jq in range(NQT):
                    for jg in range(2):   # 8 c-tiles per bf16 psum buf
                        p_et = tr_pool.tile([128, 1024], BF16, tag="tr")
                        for jj in range(8):
                            jc = jg * 8 + jj
                            nc.tensor.transpose(
                                p_et[:, ts(jj, 128)],
                                E_sb[:, jc, ts(jq, 128)], ident_bf)
                        nc.vector.tensor_copy(
                            ET[:, jq, ds(jg * 1024, 1024)], p_et)

                # ---- a / b2 matmuls + epilogue per c-chunk ----
                for jch in range(NCH):
                    p_ab = ab_pool.tile([128, 1024], F32, tag="ab")
                    for jq in range(NQT):
                        nc.tensor.matmul(
                            p_ab[:, 0:512], lhsT=qryR[:, jq, :],
                            rhs=ET[:, jq, ts(jch, 512)],
                            start=(jq == 0), stop=(jq == NQT - 1))
                    for jq in range(NQT):
                        nc.tensor.matmul(
                            p_ab[:, 512:1024], lhsT=t2[:, jq, :],
                            rhs=ET[:, jq, ts(jch, 512)],
                            start=(jq == 0), stop=(jq == NQT - 1))
                    # section 2: aT (PSUM->SBUF on ScalarE, then DMA)
                    st2 = stg.tile([128, 512], F32, tag="st2")
                    nc.scalar.copy(st2, p_ab[:, 0:512])
                    nc.sync.dma_start(
                        out=out_ext[b, ds(D, D), ts(jch, 512)], in_=st2)
                    # section 3: ctxT * aT
                    st3 = stg.tile([128, 512], F32, tag="st3")
                    nc.vector.tensor_tensor(
                        out=st3, in0=ctx_sb[:, ts(jch, 512)],
                        in1=p_ab[:, 0:512], op=mybir.AluOpType.mult)
                    nc.sync.dma_start(
                        out=out_ext[b, ds(2 * D, D), ts(jch, 512)], in_=st3)
                    # section 4: ctxT * b2T
                    st4 = stg.tile([128, 512], F32, tag="st4")
                    nc.vector.tensor_tensor(
                        out=st4, in0=ctx_sb[:, ts(jch, 512)],
                        in1=p_ab[:, 512:1024], op=mybir.AluOpType.mult)
                    nc.sync.dma_start(
                        out=out_ext[b, ds(3 * D, D), ts(jch, 512)], in_=st4)
    _split_multi_waits(nc)
    return nc


_NC = None


def kernel(context: np.ndarray, query: np.ndarray, w: np.ndarray,
           **extra) -> np.ndarray:
    global _NC
    if _NC is None:
        _NC = build_kernel()
    context = np.ascontiguousarray(context, dtype=np.float32)
    query = np.ascontiguousarray(query, dtype=np.float32)
    w = np.ascontiguousarray(w, dtype=np.float32)
    in_maps = []
    for i in range(NCORES):
        sl = slice(i * BPC, (i + 1) * BPC)
        in_maps.append({
            "context": context[sl],
            "query": query[sl],
            "w": w,
        })
    res = run_bass_kernel_spmd(_NC, in_maps, core_ids=list(range(NCORES)))
    return np.concatenate([r["out"] for r in res.results], axis=0)


if __name__ == "__main__":
    rng = np.random.default_rng(0)
    out = kernel(
        context=rng.standard_normal((B, D, C), dtype=np.float32),
        query=rng.standard_normal((B, D, Q), dtype=np.float32),
        w=(rng.random(3 * D, dtype=np.float32) - 0.5) * 2 / np.sqrt(D),
    )
    print(out.shape, out.dtype)
